# revision 8
# baseline (speedup 1.0000x reference)
"""AuxSpatialGather (per-class masked mean pooling) Trainium2 kernel.

Computes, per sample b:  ctx[k, c] = mean over pixels n with gt[n]==k of feats[c, n]
(classes with zero pixels get 0), returned as [B, C, K, 1] float32.

Strategy (8 NeuronCores, data-parallel over batch, 2 samples/core):
  - feats arrive channel-major [C, HW]. They are DVE-cast fp32->fp16 right
    behind the HBM loads, PE-transposed in plain f16 128x128 tiles (1
    cyc/row) to pixel-major, and reduced by a one-hot matmul ([128 pix, 19]
    stationary x [128 pix, 512 chan] moving, fp32 PSUM accumulation). Only
    precision loss: fp16 input quantization.
  - engines are kept decoupled: feat DMAs (2MB, 16KB partition lines) are
    issued a chunk ahead on the SP HWDGE ring, while their f16 casts are
    emitted interleaved into the previous chunk's group loop at the point
    matching each DMA's arrival. DVE executes in order, so emitting a cast
    too early would head-of-line-block the PSUM evacuations behind it and
    starve both the PE and the DMA ring (stage-slot WAR waits).
  - pixel groups are natural contiguous 128-pixel blocks: gt loads with one
    contiguous DMA per sample ([128, 128], 512B lines) and is PE-transposed
    on chip to [pixel-in-block, block], from which 19 is_equal planes give
    the one-hot weights W[p, t, k] directly.
  - the final chunk is quarter-split (ci-major within each quarter) so the
    PE tail after the last HBM byte is ~8 groups of work; the context is
    scaled by 1/max(cnt,1) and stored as [K, C] (host transposes to [C, K]).
"""

import numpy as np

NUM_CLASSES = 19
B, C, H, W = 16, 512, 128, 128
HW = H * W
N_CORES = 8
S = B // N_CORES  # samples per core
P = 128  # partitions

_compiled = None


def _build_nc(s=S, c=C, hw=HW, qw=4096):
    from concourse import bacc, mybir
    from concourse.tile import TileContext
    from concourse.masks import make_identity

    f32 = mybir.dt.float32
    f16 = mybir.dt.float16
    i32 = mybir.dt.int32
    K = NUM_CLASSES
    n_ci = c // P  # channel tiles (4)
    n_q = hw // qw  # chunks per sample (4)
    n_g = qw // P  # pixel groups per chunk (32)
    n_t = hw // P  # pixel groups per sample (128)
    n_u = 4  # DMA quarter-splits for the final chunk
    uw = qw // n_u  # columns per split (1024)

    nc = bacc.Bacc("TRN2", target_bir_lowering=False)
    feats = nc.dram_tensor("feats", [s, c, hw], f32, kind="ExternalInput")
    gt = nc.dram_tensor("gt_seg_map", [s, hw], i32, kind="ExternalInput")
    # [K, c] per sample; the host transposes to [c, K]
    out = nc.dram_tensor("out", [s, K, c], f32, kind="ExternalOutput")

    with TileContext(nc) as tc:
        with (
            tc.tile_pool(name="const", bufs=1) as const_pool,
            tc.tile_pool(name="stage", bufs=7) as stage_pool,
            tc.tile_pool(name="chunks", bufs=8) as chunk_pool,
            tc.tile_pool(name="fts", bufs=4) as fts_pool,
            tc.tile_pool(name="planes", bufs=2) as plane_pool,
            tc.tile_pool(name="small", bufs=2) as small_pool,
            tc.tile_pool(name="ftp", bufs=4, space="PSUM") as ftp_pool,
            tc.tile_pool(name="accp", bufs=2, space="PSUM") as acc_pool,
            tc.tile_pool(name="tinyp", bufs=2, space="PSUM") as tiny_pool,
        ):
            ident32 = const_pool.tile([P, P], f32)
            make_identity(nc, ident32[:])
            ident16 = const_pool.tile([P, P], f16)
            make_identity(nc, ident16[:])
            ones16 = const_pool.tile([P, 1], f16)
            nc.vector.memset(ones16[:], 1.0)

            def issue_dmas(si, q, split=False):
                """Feat loads for chunk (si, q) on the SP HWDGE ring. Default:
                one 2MB DMA per channel tile, ci order. split (final chunk):
                0.5MB quarters, ci-major within each quarter, so each batch
                of 8 pixel groups unlocks ~1.5us after its quarter starts."""
                sts = [
                    stage_pool.tile([P, qw], f32, name="st") for _ in range(n_ci)
                ]
                chs = [
                    chunk_pool.tile([P, qw], f16, name="ch") for _ in range(n_ci)
                ]
                if split:
                    for u in range(n_u):
                        for ci in range(n_ci):
                            nc.sync.dma_start(
                                out=sts[ci][:, u * uw : (u + 1) * uw],
                                in_=feats[
                                    si,
                                    ci * P : (ci + 1) * P,
                                    q * qw + u * uw : q * qw + (u + 1) * uw,
                                ],
                            )
                else:
                    for ci in range(n_ci):
                        nc.sync.dma_start(
                            out=sts[ci][:],
                            in_=feats[si, ci * P : (ci + 1) * P, q * qw : (q + 1) * qw],
                        )
                return sts, chs

            def build_planes(si):
                """One-hot planes [p, (k t)] f16 for sample si: contiguous gt
                DMA (second HWDGE ring), i32->f32, PE transpose, 19 is_equal."""
                gt_nat = plane_pool.tile([P, n_t], i32, name="gt_nat")
                nc.scalar.dma_start(
                    out=gt_nat[:], in_=gt[si].rearrange("(p t) -> p t", p=P)
                )
                gt_f = plane_pool.tile([P, n_t], f32, name="gt_f")
                nc.vector.tensor_copy(gt_f[:], gt_nat[:])
                gtT_ps = tiny_pool.tile([P, n_t], f32, name="gtT_ps", tag="tiny")
                nc.tensor.transpose(gtT_ps[:], gt_f[:], ident32[:])
                gtT = plane_pool.tile([P, n_t], f32, name="gtT")
                nc.vector.tensor_copy(gtT[:], gtT_ps[:])
                planes = plane_pool.tile([P, K * n_t], f16, name="planes")
                for k in range(K):
                    nc.vector.tensor_scalar(
                        planes[:, k * n_t : (k + 1) * n_t],
                        gtT[:],
                        float(k),
                        None,
                        op0=mybir.AluOpType.is_equal,
                    )
                return planes

            def build_recip(planes):
                """Per-class counts -> reciprocal [K, 1]."""
                partial = small_pool.tile([P, K], f32, name="partial")
                nc.vector.tensor_reduce(
                    partial[:],
                    planes[:].rearrange("p (k t) -> p k t", k=K),
                    axis=mybir.AxisListType.X,
                    op=mybir.AluOpType.add,
                )
                partial16 = small_pool.tile([P, K], f16, name="partial16")
                nc.vector.tensor_copy(partial16[:], partial[:])
                cnt_ps = tiny_pool.tile([1, K], f32, name="cnt_ps", tag="tiny")
                nc.tensor.matmul(
                    cnt_ps[:], ones16[:], partial16[:], start=True, stop=True
                )
                cnt_sq = small_pool.tile([32, 32], f32, name="cnt_sq")
                nc.vector.memset(cnt_sq[:], 0.0)
                nc.vector.tensor_copy(cnt_sq[:1, :K], cnt_ps[:])
                cnt_tr = small_pool.tile([32, 32], f32, name="cnt_tr")
                nc.vector.transpose(cnt_tr[:], cnt_sq[:])
                recip = small_pool.tile([K, 1], f32, name="recip")
                nc.vector.tensor_scalar_max(recip[:], cnt_tr[:K, :1], 1.0)
                nc.vector.reciprocal(recip[:], recip[:])
                return recip

            n_chunks = s * n_q
            LAST = n_chunks - 1

            def cast_ci(pend, ci):
                sts, chs = pend
                nc.vector.tensor_copy(chs[ci][:], sts[ci][:])

            def cast_quarter(pend, u, ci):
                sts, chs = pend
                sl = slice(u * uw, (u + 1) * uw)
                nc.vector.tensor_copy(chs[ci][:, sl], sts[ci][:, sl])

            # gt+planes first (tiny DMA on the ACT ring), then feat chunk 0;
            # chunk 0's casts are emitted immediately (nothing to block).
            planes_cur = build_planes(0)
            pending = issue_dmas(0, 0)
            for ci in range(n_ci):
                cast_ci(pending, ci)

            for si in range(s):
                acc = acc_pool.tile([K, c], f32, name="acc")
                W_all = planes_cur[:].rearrange("p (k t) -> p t k", k=K)
                for q in range(n_q):
                    idx = si * n_q + q
                    chs = pending[1]
                    nxt = None
                    if q + 1 < n_q:
                        nxt = issue_dmas(si, q + 1, split=(idx + 1 == LAST))
                    elif si + 1 < s:
                        nxt = issue_dmas(si + 1, 0)
                        planes_next = build_planes(si + 1)
                    if q == 0:
                        recip = build_recip(planes_cur)
                    for g in range(n_g):
                        # final chunk: its own casts ride ahead of each
                        # 8-group batch (quarter granularity)
                        if idx == LAST and g % 8 == 0:
                            for ci in range(n_ci):
                                cast_quarter(pending, g // 8, ci)
                        t = q * n_g + g
                        ftp = ftp_pool.tile([P, c], f16, name="ftp", tag="ftp")
                        for ci in range(n_ci):
                            nc.tensor.transpose(
                                ftp[:, ci * P : (ci + 1) * P],
                                chs[ci][:, g * P : (g + 1) * P],
                                ident16[:],
                            )
                        fts = fts_pool.tile([P, c], f16, name="fts")
                        if g % 2 == 1:
                            nc.vector.tensor_copy(fts[:], ftp[:])
                        else:
                            nc.scalar.copy(fts[:], ftp[:])
                        nc.tensor.matmul(
                            acc[:],
                            W_all[:, t, :],
                            fts[:],
                            start=(t == 0),
                            stop=(t == n_t - 1),
                        )
                        # next chunk's casts, emitted at the point in DVE
                        # program order matching each DMA's expected arrival
                        if nxt is not None and idx + 1 != LAST and g % 8 == 7:
                            cast_ci(nxt, g // 8)
                    pending = nxt

                # ---- normalize + emit [K, c] ----
                final = small_pool.tile([K, c], f32, name="final")
                nc.vector.tensor_scalar(
                    final[:], acc[:], recip[:, :1], None,
                    op0=mybir.AluOpType.mult,
                )
                # mid-stream store goes SWDGE (keeps the HWDGE rings free of
                # DMAs that wait on compute); the final store rides the idle
                # ACT HWDGE ring for its ~0.6us first-byte latency
                store_eng = nc.scalar if si == s - 1 else nc.gpsimd
                store_eng.dma_start(out=out[si], in_=final[:])
                if si + 1 < s:
                    planes_cur = planes_next
    nc.compile()
    return nc


def _get_compiled():
    global _compiled
    if _compiled is None:
        _compiled = _build_nc()
    return _compiled


def kernel(feats, gt_seg_map):
    from concourse.bass_utils import run_bass_kernel_spmd

    feats = np.asarray(feats, dtype=np.float32).reshape(B, C, HW)
    gt = np.asarray(gt_seg_map).astype(np.int32).reshape(B, HW)

    nc = _get_compiled()
    in_maps = []
    for i in range(N_CORES):
        in_maps.append(
            {
                "feats": feats[i * S : (i + 1) * S],
                "gt_seg_map": gt[i * S : (i + 1) * S],
            }
        )
    res = run_bass_kernel_spmd(nc, in_maps, core_ids=list(range(N_CORES)))
    parts = [res.results[i]["out"] for i in range(N_CORES)]  # each [S, K, C]
    full = np.concatenate(parts, axis=0)  # [B, K, C]
    return np.ascontiguousarray(np.transpose(full, (0, 2, 1)))[..., None].astype(
        np.float32
    )  # [B, C, K, 1]
